# revision 1
# baseline (speedup 1.0000x reference)
"""Multi-head attention (B=2, N=2048, D=1024, H=16) sharded over 8 trn2 cores.

Sharding: batch (2) x head-groups (4 groups of 4 heads) = 8 cores.
Each core computes, for its (batch b, head-group g):
  Q.T/K.T feature-major and V token-major projections of its group,
  S.T = K @ Q.T scores (keys on partitions, queries on free axis),
  P.T = exp(S.T / 8)  (no max subtraction -- scores are ~N(0,1), safe in fp32),
  ctx'.T = [V | ones].T @ P.T  (ones column yields softmax denominators),
  ctx.T normalized via K=1 broadcast matmul of 1/sums,
  partial O = ctx.T.T @ w_o_g.T  (row-parallel O projection).
Host sums the 4 group partials per batch and adds b_o.

Matmul operands are bf16 (host pre-converts inputs; fp32 PSUM accumulation);
biases are applied in fp32 during PSUM eviction.
"""

import os
import sys

for _p in ("/opt/trn_rl_repo",):
    if _p not in sys.path and os.path.isdir(_p):
        sys.path.insert(0, _p)

import ml_dtypes
import numpy as np

import concourse.bass as bass
import concourse.tile as tile
from concourse import bacc, mybir
from concourse.bass_utils import run_bass_kernel_spmd

F32 = mybir.dt.float32
BF16 = mybir.dt.bfloat16
EXP = mybir.ActivationFunctionType.Exp

B = 2
D = 1024
N_HEADS = 16
DK = 64
N_CORES = 8
N_GROUPS = 4  # head groups (4 heads each) across cores within a batch
GF = D // N_GROUPS  # 256 features per group
HPG = N_HEADS // N_GROUPS  # 4 heads per group
PAIRS = HPG // 2  # head pairs (2 heads of 64 feats = 128 partitions)
KC = D // 128  # contraction chunks for the input projections


def build_nc(n_tok: int, loop_k: int = 1):
    """Build the single-core Bass program (same program for all 8 cores).

    Software-pipelined emission order: V-projection + K-pair0 + Q-chunk0 as
    prefix, then attention chains (p outer, qc inner) with the remaining
    K/Q projection slices interleaved between chains so the PE fills its
    ACT-paced stalls. loop_k > 1 wraps the body in a For_i for timing.
    """
    import contextlib
    assert n_tok % 512 == 0
    QC = n_tok // 512  # query chunks of 512
    TT = n_tok // 128  # token (and key) tiles of 128

    nc = bacc.Bacc("TRN2", target_bir_lowering=False, debug=False,
                   num_devices=N_CORES)

    xqT = nc.dram_tensor("xqT", [D, n_tok], BF16, kind="ExternalInput")
    xkT = nc.dram_tensor("xkT", [D, n_tok], BF16, kind="ExternalInput")
    xvT = nc.dram_tensor("xvT", [D, n_tok], BF16, kind="ExternalInput")
    wqT = nc.dram_tensor("wqT", [D, GF], BF16, kind="ExternalInput")
    wkT = nc.dram_tensor("wkT", [D, GF], BF16, kind="ExternalInput")
    wvT = nc.dram_tensor("wvT", [D, GF], BF16, kind="ExternalInput")
    woT = nc.dram_tensor("woT", [GF, D], BF16, kind="ExternalInput")
    bq2 = nc.dram_tensor("bq2", [128, 2], F32, kind="ExternalInput")
    bk2 = nc.dram_tensor("bk2", [128, 2], F32, kind="ExternalInput")
    bvp = nc.dram_tensor("bvp", [1, GF], BF16, kind="ExternalInput")
    out_p = nc.dram_tensor("out_p", [n_tok, D], BF16, kind="ExternalOutput")

    def mm(out, lhsT, rhs, **kw):
        nc.tensor.matmul(out, lhsT, rhs, **kw)

    with tile.TileContext(nc) as tc:
      with (tc.For_i(0, loop_k, 1) if loop_k > 1
            else contextlib.nullcontext()):
        with (
            tc.tile_pool(name="weights", bufs=1) as wpool,
            tc.tile_pool(name="acts", bufs=1) as apool,
            tc.tile_pool(name="xs", bufs=1) as xspool,
        ):
            wq_sb = wpool.tile([128, KC * GF], BF16, tag="wq")
            wk_sb = wpool.tile([128, KC * GF], BF16, tag="wk")
            wv_sb = wpool.tile([128, KC * GF], BF16, tag="wv")
            wo_sb = wpool.tile([128, 2 * D], BF16, tag="wo")
            bq_sb = wpool.tile([128, 2], F32, tag="bq")
            bk_sb = wpool.tile([128, 2], F32, tag="bk")
            bvp_sb = wpool.tile([1, GF], BF16, tag="bvp")
            ones_sb = wpool.tile([128, 128], BF16, tag="ones")

            # DMA emission order = need order: tiny biases, then K stream,
            # Q stream, V stream, O weights (the DMA device is the per-core
            # HBM bandwidth; later tensors land later).
            nc.sync.dma_start(bq_sb[:], bq2[:])
            nc.sync.dma_start(bk_sb[:], bk2[:])
            nc.sync.dma_start(bvp_sb[:], bvp[:])
            nc.vector.memset(ones_sb[:], 1.0)
            # tiny dummy exp: forces the ACT exp-table load at t~0,
            # off the first-chain critical path
            warm_sb = wpool.tile([1, 8], F32, tag="warm")
            nc.scalar.activation(warm_sb[:], ones_sb[0:1, 0:8], EXP)

            # Q.T / K.T feature-major [2 pair-tiles x 128, n_tok]
            qt_sb = apool.tile([128, PAIRS * n_tok], BF16, tag="qt")
            kt_sb = apool.tile([128, PAIRS * n_tok], BF16, tag="kt")
            # V' token-major with per-head ones column: [n_tok, HPG*65]
            v_sb = apool.tile([128, TT * HPG * 65], BF16, tag="v")
            nc.vector.memset(v_sb[:], 1.0)

            xk_t, xv_t, xq_t = [], [], []
            for pfx, lst in (("xk", xk_t), ("xv", xv_t), ("xq", xq_t)):
                for k in range(KC):
                    t = xspool.tile([128, n_tok], BF16, tag=f"{pfx}{k}",
                                    name=f"{pfx}{k}")
                    lst.append(t)

            def load_w(w_dram, w_sb):
                nc.sync.dma_start(
                    w_sb[:].rearrange("p (k f) -> p k f", f=GF),
                    w_dram[:].rearrange("(k p) f -> p k f", p=128))

            def load_x(lst, x_dram):
                for k in range(KC):
                    nc.sync.dma_start(lst[k][:],
                                      x_dram[k * 128:(k + 1) * 128, :])

            def load_slab(lst, x_dram, b):
                for k in range(KC):
                    nc.sync.dma_start(lst[k][:, b * 512:(b + 1) * 512],
                                      x_dram[b, k * 128:(k + 1) * 128, :])

            # arrival order = need order: K stream, V stream, Q stream, wo
            load_w(wkT, wk_sb)
            load_x(xk_t, xkT)
            load_w(wvT, wv_sb)
            load_x(xv_t, xvT)
            load_w(wqT, wq_sb)
            load_x(xq_t, xqT)
            nc.sync.dma_start(
                wo_sb[:].rearrange("p (c f) -> p c f", f=D),
                woT[:].rearrange("(c p) f -> p c f", p=128),
            )

            def v_slice(pool, tag, t):
                vp = pool.tile([128, GF], F32, tag=tag, name="vp")
                mm(vp[:], ones_sb[0:1, 0:128], bvp_sb[0:1, :],
                   start=True, stop=False)
                for k in range(KC):
                    mm(vp[:],
                       xv_t[k][:, t * 128:(t + 1) * 128],
                       wv_sb[:, k * GF:(k + 1) * GF],
                       start=False, stop=(k == KC - 1))
                dst = v_sb[:, t * HPG * 65:(t + 1) * HPG * 65]
                nc.vector.tensor_copy(
                    dst.rearrange("p (h c) -> p h c", c=65)[:, :, 0:DK],
                    vp[:].rearrange("p (h c) -> p h c", c=DK))

            def proj_slice(pool, tag, xt, w_sb, b_sb, dst, qc, m):
                ps = pool.tile([128, 512], F32, tag=tag, name="pqk")
                for k in range(KC):
                    mm(ps[:],
                       w_sb[:, k * GF + m * 128:k * GF + (m + 1) * 128],
                       xt[k][:, qc * 512:(qc + 1) * 512],
                       start=(k == 0), stop=(k == KC - 1))
                nc.vector.tensor_scalar_add(
                    dst[:, m * n_tok + qc * 512:m * n_tok + (qc + 1) * 512],
                    ps[:], b_sb[:, m:m + 1])

            # ---- prefix: V complete, K pair0 (all qc), Q (qc0, pair0) ----
            # k-major prefix: matmuls chase the arriving xk/xq chunks
            with tc.tile_pool(name="psqk", bufs=5, space="PSUM") as psqk:
                kps = [psqk.tile([128, 512], F32, tag="pqk", name=f"kps{qc}")
                       for qc in range(QC)]
                for k in range(KC):
                    for qc in range(QC):
                        mm(kps[qc][:],
                           wk_sb[:, k * GF:k * GF + 128],
                           xk_t[k][:, qc * 512:(qc + 1) * 512],
                           start=(k == 0), stop=(k == KC - 1))
                for qc in range(QC):
                    nc.vector.tensor_scalar_add(
                        kt_sb[:, qc * 512:(qc + 1) * 512],
                        kps[qc][:], bk_sb[:, 0:1])
                # V' k-major in 4-token-tile blocks (chases xv slabs)
                for blk in range(0, TT, 4):
                    nt = min(4, TT - blk)
                    vps = [psqk.tile([128, GF], F32, tag="pqk",
                                     name=f"vps{t}") for t in range(nt)]
                    for t in range(nt):
                        mm(vps[t][:], ones_sb[0:1, 0:128], bvp_sb[0:1, :],
                           start=True, stop=False)
                    for k in range(KC):
                        for t in range(nt):
                            tt = blk + t
                            mm(vps[t][:],
                               xv_t[k][:, tt * 128:(tt + 1) * 128],
                               wv_sb[:, k * GF:(k + 1) * GF],
                               start=False, stop=(k == KC - 1))
                    for t in range(nt):
                        tt = blk + t
                        dst = v_sb[:, tt * HPG * 65:(tt + 1) * HPG * 65]
                        # ACT (idle in the prefix) does these so the DVE FIFO
                        # stays clear for the K/Q evictions the first scores
                        # depend on
                        nc.scalar.copy(
                            dst.rearrange("p (h c) -> p h c", c=65)[:, :, 0:DK],
                            vps[t][:].rearrange("p (h c) -> p h c", c=DK))
                qps = psqk.tile([128, 512], F32, tag="pqk", name="qps")
                for k in range(KC):
                    mm(qps[:],
                       wq_sb[:, k * GF:k * GF + 128],
                       xq_t[k][:, 0:512],
                       start=(k == 0), stop=(k == KC - 1))
                nc.vector.tensor_scalar_add(
                    qt_sb[:, 0:512], qps[:], bq_sb[:, 0:1])

            # ---- attention + O projection, with remaining projection and
            # O slices interleaved one per kt2 group (PE fills ACT stalls) --
            with (
                tc.tile_pool(name="pt", bufs=6) as ptpool,
                tc.tile_pool(name="rcp", bufs=2) as rcpool,
                tc.tile_pool(name="ctx", bufs=1) as ctxpool,
                tc.tile_pool(name="ost", bufs=4) as opool,
                tc.tile_pool(name="psctx", bufs=2, space="PSUM") as psctx,
                tc.tile_pool(name="pss", bufs=2, space="PSUM") as pss,
                tc.tile_pool(name="psbo", bufs=2, space="PSUM") as psbo,
            ):
                ctx_t = {}

                deferred = []
                k0_work = []
                pk_work = [("k", qc, 1) for qc in range(QC)]
                pk_work += [("q", qc, 0) for qc in range(1, QC)]
                pk_work += [("q", qc, 1) for qc in range(QC)]
                o_work = []

                def emit_pk(item):
                    kind, qc_, m_ = item
                    if kind == "q":
                        proj_slice(psbo, "bo", xq_t, wq_sb, bq_sb,
                                   qt_sb, qc_, m_)
                    else:
                        proj_slice(psbo, "bo", xk_t, wk_sb, bk_sb,
                                   kt_sb, qc_, m_)

                def emit_o(item):
                    qc_, s_ = item
                    ost = opool.tile([128, D], BF16, tag="ost", name="ost")
                    for n in range(2):
                        op = psbo.tile([128, 512], F32, tag="bo", name="op")
                        for cp in range(PAIRS):
                            mm(op[:],
                               ctx_t[(cp, qc_)][:, s_ * 128:(s_ + 1) * 128],
                               wo_sb[:, cp * D + n * 512:cp * D + (n + 1) * 512],
                               start=(cp == 0), stop=(cp == PAIRS - 1))
                        nc.vector.tensor_copy(
                            ost[:, n * 512:(n + 1) * 512], op[:])
                    nc.sync.dma_start(
                        out_p[qc_ * 512 + s_ * 128:qc_ * 512 + (s_ + 1) * 128, :],
                        ost[:])

                def emit_one():
                    if o_work:
                        emit_o(o_work.pop(0))
                    elif pk_work:
                        emit_pk(pk_work.pop(0))

                def force_deadline(qc, p):
                    for item in [i for i in pk_work
                                 if i[0] == "q" and i[1] == qc and i[2] == p]:
                        pk_work.remove(item)
                        emit_pk(item)
                    if p == 1:
                        for item in [i for i in pk_work if i[0] == "k"]:
                            pk_work.remove(item)
                            emit_pk(item)

                for qc in range(QC):
                    for p in range(PAIRS):
                        force_deadline(qc, p)
                        ctx_t[(p, qc)] = ctxpool.tile(
                            [128, 512], BF16, tag=f"ctx{p}{qc}",
                            name=f"ctx{p}{qc}")
                        cps = [psctx.tile([128, 512], F32, tag="cps",
                                          name=f"cps{h}")
                               for h in range(2)]
                        for kt2 in range(TT // 2):
                            s2 = [pss.tile([128, 1024], F32, tag="s",
                                           name=f"s2_{h}")
                                  for h in range(2)]
                            for half in range(2):
                                kt = 2 * kt2 + half
                                for h in range(2):
                                    mm(s2[h][:, half * 512:(half + 1) * 512],
                                       kt_sb[64 * h:64 * h + 64,
                                             p * n_tok + kt * 128:
                                             p * n_tok + (kt + 1) * 128],
                                       qt_sb[64 * h:64 * h + 64,
                                             p * n_tok + qc * 512:
                                             p * n_tok + (qc + 1) * 512],
                                       start=True, stop=True)
                            pt = []
                            for h in range(2):
                                ptile = ptpool.tile([128, 1024], BF16,
                                                    tag="pt")
                                nc.scalar.activation(ptile[:], s2[h][:], EXP,
                                                     scale=1.0 / np.sqrt(DK))
                                pt.append(ptile)
                            if kt2 == 0:
                                for cl in deferred:
                                    cl()
                                deferred.clear()
                            for half in range(2):
                                kt = 2 * kt2 + half
                                for h in range(2):
                                    hh = 2 * p + h
                                    mm(cps[h][0:65, :],
                                       v_sb[:, kt * HPG * 65 + hh * 65:
                                               kt * HPG * 65 + (hh + 1) * 65],
                                       pt[h][:, half * 512:(half + 1) * 512],
                                       start=(kt == 0), stop=(kt == TT - 1))
                            if k0_work:
                                # K pair0 qc-slices, just ahead of the groups
                                # whose scores read them (group g needs qc g//2)
                                while k0_work and k0_work[0] <= (kt2 + 2) // 2:
                                    proj_slice(psbo, "bo", xk_t, wk_sb, bk_sb,
                                               kt_sb, k0_work.pop(0), 0)
                            else:
                                emit_one()
                        while k0_work:
                            proj_slice(psbo, "bo", xk_t, wk_sb, bk_sb,
                                       kt_sb, k0_work.pop(0), 0)

                        def make_norm(p_, qc_, cps_):
                            def norm():
                                for h in range(2):
                                    rc = rcpool.tile([128, 512], BF16,
                                                     tag="rc", name="rc")
                                    with nc.allow_low_precision(
                                            reason="bf16 recip -> bcast mm"):
                                        nc.vector.reciprocal(
                                            rc[64:65, :], cps_[h][64:65, :])
                                    bp = psbo.tile([128, 512], F32, tag="bo",
                                                   name="bp")
                                    mm(bp[0:64, :], ones_sb[64:65, 0:64],
                                       rc[64:65, :], start=True, stop=True)
                                    bb = rcpool.tile([128, 512], F32,
                                                     tag="bb", name="bb")
                                    nc.vector.tensor_copy(bb[0:64, :],
                                                          bp[0:64, :])
                                    nc.vector.tensor_mul(
                                        ctx_t[(p_, qc_)][64 * h:64 * h + 64, :],
                                        cps_[h][0:64, :], bb[0:64, :])
                                if p_ == PAIRS - 1:
                                    o_work.extend([(qc_, s_)
                                                   for s_ in range(4)])
                            return norm

                        # defer the tail: it is emitted inside the NEXT
                        # chain's first group, behind that chain's first
                        # scores/exp, so ACT never idles at the boundary
                        deferred.append(make_norm(p, qc, cps))
                for cl in deferred:
                    cl()
                deferred.clear()
                while o_work or pk_work:
                    emit_one()

    nc.compile()
    return nc


_NC_CACHE: dict[int, object] = {}


def get_nc(n_tok: int):
    if n_tok not in _NC_CACHE:
        _NC_CACHE[n_tok] = build_nc(n_tok)
    return _NC_CACHE[n_tok]


def make_in_maps(query, key, value, w_q, b_q, w_k, b_k, w_v, b_v, w_o, b_o):
    n_tok = query.shape[1]
    bf16 = ml_dtypes.bfloat16
    xT = {}
    for b in range(B):
        xT[("q", b)] = np.ascontiguousarray(query[b].T.astype(bf16))
        xT[("k", b)] = np.ascontiguousarray(key[b].T.astype(bf16))
        xT[("v", b)] = np.ascontiguousarray(value[b].T.astype(bf16))
    in_maps = []
    for core in range(N_CORES):
        b, g = divmod(core, N_GROUPS)
        gs = slice(g * GF, (g + 1) * GF)
        in_maps.append({
            "xqT": xT[("q", b)],
            "xkT": xT[("k", b)],
            "xvT": xT[("v", b)],
            "wqT": np.ascontiguousarray(w_q[gs, :].T.astype(bf16)),
            "wkT": np.ascontiguousarray(w_k[gs, :].T.astype(bf16)),
            "wvT": np.ascontiguousarray(w_v[gs, :].T.astype(bf16)),
            "woT": np.ascontiguousarray(w_o[:, gs].T.astype(bf16)),
            "bq2": np.ascontiguousarray(
                b_q[gs].reshape(2, 128).T, np.float32),
            "bk2": np.ascontiguousarray(
                b_k[gs].reshape(2, 128).T, np.float32),
            "bvp": np.ascontiguousarray(b_v[gs].reshape(1, GF).astype(bf16)),
        })
    return in_maps


def kernel(**inputs):
    query = np.asarray(inputs["query"], np.float32)
    n_tok = query.shape[1]
    nc = get_nc(n_tok)
    in_maps = make_in_maps(
        query, np.asarray(inputs["key"], np.float32),
        np.asarray(inputs["value"], np.float32),
        np.asarray(inputs["w_q"], np.float32), np.asarray(inputs["b_q"], np.float32),
        np.asarray(inputs["w_k"], np.float32), np.asarray(inputs["b_k"], np.float32),
        np.asarray(inputs["w_v"], np.float32), np.asarray(inputs["b_v"], np.float32),
        np.asarray(inputs["w_o"], np.float32), np.asarray(inputs["b_o"], np.float32),
    )
    res = run_bass_kernel_spmd(nc, in_maps, core_ids=list(range(N_CORES)))
    out = np.zeros((B, n_tok, D), np.float32)
    for core in range(N_CORES):
        b = core // N_GROUPS
        out[b] += res.results[core]["out_p"].astype(np.float32)
    out += np.asarray(inputs["b_o"], np.float32)
    return out



# revision 6
# speedup vs baseline: 1.9266x; 1.9266x over previous
"""Multi-head attention (B=2, N=2048, D=1024, H=16) sharded over 8 trn2 cores.

Sharding: batch (2) x head-groups (4 groups of 4 heads) = 8 cores.
Each core computes, for its (batch b, head-group g):
  Q.T/K.T feature-major and V token-major projections of its group,
  S.T = K @ Q.T scores (keys on partitions, queries on free axis),
  P.T = exp(S.T / 8)  (no max subtraction -- scores are ~N(0,1), safe in fp32),
  ctx'.T = [V | ones].T @ P.T  (ones column yields softmax denominators),
  ctx.T normalized via K=1 broadcast matmul of 1/sums,
  partial O = ctx.T.T @ w_o_g.T  (row-parallel O projection).
Host sums the 4 group partials per batch and adds b_o.

Schedule (v2): DMA order wk,xk,wq,xq,wv,xv,wo. K then Q projections run
k-major in the prefix, chasing DMA chunk arrivals across all 8 PSUM banks
(scoped pool, released before the chains). Attention chains start as soon
as Q(qc0) lands (~31us): per kt2 group scores pair-matmuls (row-tiled
(0,0)/(64,0) run concurrently on HW) then one exp per head on ACT. The
attn@V (ctx) consumption of chain c is emitted during chain c+1 (one-chain
lag, pt pool holds 32 exp tiles) so ACT never waits on the V projection,
which streams tile-major through chains 1-2 after xv lands. O-projection
slabs fill remaining slots. Matmul operands bf16 (fp32 PSUM accumulation);
biases applied in fp32 during PSUM eviction.
"""

import os
import sys

for _p in ("/opt/trn_rl_repo",):
    if _p not in sys.path and os.path.isdir(_p):
        sys.path.insert(0, _p)

import ml_dtypes
import numpy as np

import concourse.bass as bass
import concourse.tile as tile
from concourse import bacc, mybir
from concourse.bass_utils import run_bass_kernel_spmd

F32 = mybir.dt.float32
BF16 = mybir.dt.bfloat16
EXP = mybir.ActivationFunctionType.Exp

B = 2
D = 1024
N_HEADS = 16
DK = 64
N_CORES = 8
N_GROUPS = 4  # head groups (4 heads each) across cores within a batch
GF = D // N_GROUPS  # 256 features per group
HPG = N_HEADS // N_GROUPS  # 4 heads per group
PAIRS = HPG // 2  # head pairs (2 heads of 64 feats = 128 partitions)
KC = D // 128  # contraction chunks for the input projections


def build_nc(n_tok: int, loop_k: int = 1):
    """Build the single-core Bass program (same program for all 8 cores)."""
    import contextlib
    assert n_tok % 512 == 0
    QC = n_tok // 512  # query chunks of 512
    TT = n_tok // 128  # token (and key) tiles of 128

    nc = bacc.Bacc("TRN2", target_bir_lowering=False, debug=False,
                   num_devices=N_CORES)

    xqT = nc.dram_tensor("xqT", [D, n_tok], BF16, kind="ExternalInput")
    xkT = nc.dram_tensor("xkT", [D, n_tok], BF16, kind="ExternalInput")
    xvT = nc.dram_tensor("xvT", [D, n_tok], BF16, kind="ExternalInput")
    wqT = nc.dram_tensor("wqT", [D, GF], BF16, kind="ExternalInput")
    wkT = nc.dram_tensor("wkT", [D, GF], BF16, kind="ExternalInput")
    wvT = nc.dram_tensor("wvT", [D, GF], BF16, kind="ExternalInput")
    woT = nc.dram_tensor("woT", [GF, D], BF16, kind="ExternalInput")
    bq2 = nc.dram_tensor("bq2", [128, 2], F32, kind="ExternalInput")
    bk2 = nc.dram_tensor("bk2", [128, 2], F32, kind="ExternalInput")
    bvp = nc.dram_tensor("bvp", [1, GF], BF16, kind="ExternalInput")
    out_p = nc.dram_tensor("out_p", [n_tok, D], BF16, kind="ExternalOutput")

    def mm(out, lhsT, rhs, **kw):
        nc.tensor.matmul(out, lhsT, rhs, **kw)

    with tile.TileContext(nc) as tc:
      with (tc.For_i(0, loop_k, 1) if loop_k > 1
            else contextlib.nullcontext()):
        with (
            tc.tile_pool(name="weights", bufs=1) as wpool,
            tc.tile_pool(name="acts", bufs=1) as apool,
        ):
            wq_sb = wpool.tile([128, KC * GF], BF16, tag="wq")
            wk_sb = wpool.tile([128, KC * GF], BF16, tag="wk")
            wv_sb = wpool.tile([128, KC * GF], BF16, tag="wv")
            wo_sb = wpool.tile([128, 2 * D], BF16, tag="wo")
            bq_sb = wpool.tile([128, 2], F32, tag="bq")
            bk_sb = wpool.tile([128, 2], F32, tag="bk")
            bvp_sb = wpool.tile([1, GF], BF16, tag="bvp")
            ones_sb = wpool.tile([128, 128], BF16, tag="ones")

            nc.sync.dma_start(bq_sb[:], bq2[:])
            nc.sync.dma_start(bk_sb[:], bk2[:])
            nc.sync.dma_start(bvp_sb[:], bvp[:])
            nc.vector.memset(ones_sb[:], 1.0)
            # tiny dummy exp: forces the ACT exp-table load at t~0
            warm_sb = wpool.tile([1, 8], F32, tag="warm")
            nc.scalar.activation(warm_sb[:], ones_sb[0:1, 0:8], EXP)

            # Q.T / K.T feature-major [2 pair-tiles x 128, n_tok]
            qt_sb = apool.tile([128, PAIRS * n_tok], BF16, tag="qt")
            kt_sb = apool.tile([128, PAIRS * n_tok], BF16, tag="kt")
            # V' token-major with per-head ones column: [n_tok, HPG*65]
            v_sb = apool.tile([128, TT * HPG * 65], BF16, tag="v")
            nc.vector.memset(v_sb[:], 1.0)

            def load_w(w_dram, w_sb):
                nc.sync.dma_start(
                    w_sb[:].rearrange("p (k f) -> p k f", f=GF),
                    w_dram[:].rearrange("(k p) f -> p k f", p=128))

            # ---- prefix: K then Q projections, k-major in TWO 4-slice
            # waves over PSUM banks 0-3 (wave-A evictions overlap wave-B
            # matmuls; banks 4-7 stay clean for the chains' score tiles)
            WAVES = [[(qc, m) for qc in (0, 1) for m in range(2)],
                     [(qc, m) for qc in (2, 3) for m in range(2)]]

            with (
                tc.tile_pool(name="xkq", bufs=1) as xs1,
                tc.tile_pool(name="pskq", bufs=1, space="PSUM") as ps1,
            ):
                xk_t = [xs1.tile([128, n_tok], BF16, tag=f"xk{k}",
                                 name=f"xk{k}") for k in range(KC)]
                xq_t = [xs1.tile([128, n_tok], BF16, tag=f"xq{k}",
                                 name=f"xq{k}") for k in range(KC)]
                # DMA arrival order = need order
                load_w(wkT, wk_sb)
                for k in range(KC):
                    nc.sync.dma_start(xk_t[k][:],
                                      xkT[k * 128:(k + 1) * 128, :])
                load_w(wqT, wq_sb)
                for k in range(KC):
                    nc.sync.dma_start(xq_t[k][:],
                                      xqT[k * 128:(k + 1) * 128, :])

                for (x_t, w_sb, b_sb, dst) in (
                        (xk_t, wk_sb, bk_sb, kt_sb),
                        (xq_t, wq_sb, bq_sb, qt_sb)):
                    for wave in WAVES:
                        pslices = [ps1.tile([128, 512], F32, tag=f"s{i}",
                                            name=f"ps_{i}")
                                   for i in range(len(wave))]
                        for k in range(KC):
                            for i, (qc, m) in enumerate(wave):
                                mm(pslices[i][:],
                                   w_sb[:, k * GF + m * 128:
                                           k * GF + (m + 1) * 128],
                                   x_t[k][:, qc * 512:(qc + 1) * 512],
                                   start=(k == 0), stop=(k == KC - 1))
                        for i, (qc, m) in enumerate(wave):
                            nc.vector.tensor_scalar_add(
                                dst[:, m * n_tok + qc * 512:
                                       m * n_tok + (qc + 1) * 512],
                                pslices[i][:], b_sb[:, m:m + 1])

            # ---- chains, V projection, lagged ctx, O projection ----
            with (
                tc.tile_pool(name="xv", bufs=1) as xs2,
                tc.tile_pool(name="pt", bufs=36) as ptpool,
                tc.tile_pool(name="rcp", bufs=2) as rcpool,
                tc.tile_pool(name="ctx", bufs=1) as ctxpool,
                tc.tile_pool(name="ost", bufs=4) as opool,
                tc.tile_pool(name="psctx", bufs=2, space="PSUM") as psctx,
                tc.tile_pool(name="psbo", bufs=2, space="PSUM") as psbo,
                tc.tile_pool(name="pss", bufs=2, space="PSUM") as pss,
            ):
                xv_t = [xs2.tile([128, n_tok], BF16, tag=f"xv{k}",
                                 name=f"xv{k}") for k in range(KC)]
                load_w(wvT, wv_sb)
                for k in range(KC):
                    nc.sync.dma_start(xv_t[k][:],
                                      xvT[k * 128:(k + 1) * 128, :])
                nc.sync.dma_start(
                    wo_sb[:].rearrange("p (c f) -> p c f", f=D),
                    woT[:].rearrange("(c p) f -> p c f", p=128),
                )

                def v_slice(t):
                    vp = psbo.tile([128, GF], F32, tag="bo", name="vp")
                    mm(vp[:], ones_sb[0:1, 0:128], bvp_sb[0:1, :],
                       start=True, stop=False)
                    for k in range(KC):
                        mm(vp[:],
                           xv_t[k][:, t * 128:(t + 1) * 128],
                           wv_sb[:, k * GF:(k + 1) * GF],
                           start=False, stop=(k == KC - 1))
                    dst = v_sb[:, t * HPG * 65:(t + 1) * HPG * 65]
                    nc.vector.tensor_copy(
                        dst.rearrange("p (h c) -> p h c", c=65)[:, :, 0:DK],
                        vp[:].rearrange("p (h c) -> p h c", c=DK))

                chains = [(qc, p) for qc in range(QC) for p in range(PAIRS)]
                pt_store = {}   # (ci, g) -> [pt_h0, pt_h1]
                cps_store = {}  # ci -> [cps_h0, cps_h1]
                ctx_t = {}
                vq = list(range(TT))
                lagq = []  # (v_tiles_needed, closure)
                o_work = []
                v_emitted = 0

                def ctx_group(ci, g):
                    qc, p = chains[ci]
                    if g == 0:
                        cps_store[ci] = [
                            psctx.tile([128, 512], F32, tag="cps",
                                       name=f"cps{ci}_{h}") for h in range(2)]
                    cps = cps_store[ci]
                    pts = pt_store.pop((ci, g))
                    for half in range(2):
                        kt = 2 * g + half
                        for h in range(2):
                            hh = 2 * p + h
                            mm(cps[h][0:65, :],
                               v_sb[:, kt * HPG * 65 + hh * 65:
                                       kt * HPG * 65 + (hh + 1) * 65],
                               pts[h][:, half * 512:(half + 1) * 512],
                               start=(kt == 0), stop=(kt == TT - 1))

                def norm(ci):
                    qc, p = chains[ci]
                    cps = cps_store.pop(ci)
                    ctx_t[(p, qc)] = ctxpool.tile(
                        [128, 512], BF16, tag=f"ctx{p}{qc}",
                        name=f"ctx{p}{qc}")
                    for h in range(2):
                        rc = rcpool.tile([128, 512], BF16, tag="rc",
                                         name="rc")
                        with nc.allow_low_precision(
                                reason="bf16 recip -> bcast mm"):
                            nc.vector.reciprocal(
                                rc[64:65, :], cps[h][64:65, :])
                        bp = psbo.tile([128, 512], F32, tag="bo", name="bp")
                        mm(bp[0:64, :], ones_sb[64:65, 0:64],
                           rc[64:65, :], start=True, stop=True)
                        bb = rcpool.tile([128, 512], F32, tag="bb", name="bb")
                        nc.vector.tensor_copy(bb[0:64, :], bp[0:64, :])
                        nc.vector.tensor_mul(
                            ctx_t[(p, qc)][64 * h:64 * h + 64, :],
                            cps[h][0:64, :], bb[0:64, :])
                    if p == PAIRS - 1:
                        o_work.extend([(qc, s_) for s_ in range(4)])

                def emit_o(item):
                    qc_, s_ = item
                    ost = opool.tile([128, D], BF16, tag="ost", name="ost")
                    for n in range(2):
                        op = psbo.tile([128, 512], F32, tag="bo", name="op")
                        for cp in range(PAIRS):
                            mm(op[:],
                               ctx_t[(cp, qc_)][:, s_ * 128:(s_ + 1) * 128],
                               wo_sb[:, cp * D + n * 512:cp * D + (n + 1) * 512],
                               start=(cp == 0), stop=(cp == PAIRS - 1))
                        nc.vector.tensor_copy(
                            ost[:, n * 512:(n + 1) * 512], op[:])
                    nc.sync.dma_start(
                        out_p[qc_ * 512 + s_ * 128:qc_ * 512 + (s_ + 1) * 128, :],
                        ost[:])

                # every chain's ctx consumption is lagged into the next
                # chain's slots (uniform one-chain lag: bridges the
                # V-projection window, and chain c+1's cps allocation always
                # follows norm(c) in emission order -- deadlock-free).
                INLINE_FROM = 99
                n_chains = len(chains)
                for ci, (qc, p) in enumerate(chains):
                    inline = ci >= INLINE_FROM
                    last = ci == n_chains - 1
                    self_pending = []
                    for g in range(TT // 2):
                        # scores + exp for this chain's group g
                        s2 = [pss.tile([128, 1024], F32, tag="s",
                                       name=f"s2_{h}") for h in range(2)]
                        for half in range(2):
                            kt = 2 * g + half
                            for h in range(2):
                                mm(s2[h][:, half * 512:(half + 1) * 512],
                                   kt_sb[64 * h:64 * h + 64,
                                         p * n_tok + kt * 128:
                                         p * n_tok + (kt + 1) * 128],
                                   qt_sb[64 * h:64 * h + 64,
                                         p * n_tok + qc * 512:
                                         p * n_tok + (qc + 1) * 512],
                                   start=True, stop=True)
                        pts = []
                        for h in range(2):
                            ptile = ptpool.tile([128, 1024], BF16, tag="pt")
                            nc.scalar.activation(ptile[:], s2[h][:], EXP,
                                                 scale=1.0 / np.sqrt(DK))
                            pts.append(ptile)
                        pt_store[(ci, g)] = pts
                        if inline:
                            ctx_group(ci, g)
                        elif last:
                            self_pending.append(g)
                        # fill the ACT-paced slot: V tiles first (chains 1+),
                        # then lagged ctx/norm work (guarded on the V tiles
                        # they read having been emitted), then O slabs
                        budget = 1 if inline else 2
                        while vq and ci >= 1 and budget > 0:
                            v_slice(vq.pop(0))
                            v_emitted += 1
                            budget -= 1
                        while (lagq and budget > 0
                               and lagq[0][0] <= v_emitted):
                            lagq.pop(0)[1]()
                            budget -= 1
                        # last chain: once the lag queue is empty its own
                        # ctx groups run in-slot (always after norm(c_prev),
                        # keeping the cps WAR emission order safe)
                        while (last and not lagq and self_pending
                               and budget > 0):
                            ctx_group(ci, self_pending.pop(0))
                            budget -= 1
                        if budget > 0 and not lagq and not vq and o_work:
                            emit_o(o_work.pop(0))
                    if inline:
                        norm(ci)
                    elif not last:
                        lagq.extend([
                            (2 * g2 + 2,
                             (lambda ci_=ci, g_=g2: ctx_group(ci_, g_)))
                            for g2 in range(TT // 2)])
                        lagq.append((0, lambda ci_=ci: norm(ci_)))
                # tail: drain remaining lagged ctx, the last chain's
                # remaining ctx groups, norms and O slabs
                while vq:
                    v_slice(vq.pop(0))
                    v_emitted += 1
                for _, cl in lagq:
                    cl()
                lagq.clear()
                for g2 in self_pending:
                    ctx_group(n_chains - 1, g2)
                norm(n_chains - 1)
                while o_work:
                    emit_o(o_work.pop(0))

    nc.compile()
    return nc


_NC_CACHE: dict[int, object] = {}


def get_nc(n_tok: int):
    if n_tok not in _NC_CACHE:
        _NC_CACHE[n_tok] = build_nc(n_tok)
    return _NC_CACHE[n_tok]


def make_in_maps(query, key, value, w_q, b_q, w_k, b_k, w_v, b_v, w_o, b_o):
    n_tok = query.shape[1]
    bf16 = ml_dtypes.bfloat16
    xT = {}
    for b in range(B):
        xT[("q", b)] = np.ascontiguousarray(query[b].T.astype(bf16))
        xT[("k", b)] = np.ascontiguousarray(key[b].T.astype(bf16))
        xT[("v", b)] = np.ascontiguousarray(value[b].T.astype(bf16))
    in_maps = []
    for core in range(N_CORES):
        b, g = divmod(core, N_GROUPS)
        gs = slice(g * GF, (g + 1) * GF)
        in_maps.append({
            "xqT": xT[("q", b)],
            "xkT": xT[("k", b)],
            "xvT": xT[("v", b)],
            "wqT": np.ascontiguousarray(w_q[gs, :].T.astype(bf16)),
            "wkT": np.ascontiguousarray(w_k[gs, :].T.astype(bf16)),
            "wvT": np.ascontiguousarray(w_v[gs, :].T.astype(bf16)),
            "woT": np.ascontiguousarray(w_o[:, gs].T.astype(bf16)),
            "bq2": np.ascontiguousarray(
                b_q[gs].reshape(2, 128).T, np.float32),
            "bk2": np.ascontiguousarray(
                b_k[gs].reshape(2, 128).T, np.float32),
            "bvp": np.ascontiguousarray(b_v[gs].reshape(1, GF).astype(bf16)),
        })
    return in_maps


def kernel(**inputs):
    query = np.asarray(inputs["query"], np.float32)
    n_tok = query.shape[1]
    nc = get_nc(n_tok)
    in_maps = make_in_maps(
        query, np.asarray(inputs["key"], np.float32),
        np.asarray(inputs["value"], np.float32),
        np.asarray(inputs["w_q"], np.float32), np.asarray(inputs["b_q"], np.float32),
        np.asarray(inputs["w_k"], np.float32), np.asarray(inputs["b_k"], np.float32),
        np.asarray(inputs["w_v"], np.float32), np.asarray(inputs["b_v"], np.float32),
        np.asarray(inputs["w_o"], np.float32), np.asarray(inputs["b_o"], np.float32),
    )
    res = run_bass_kernel_spmd(nc, in_maps, core_ids=list(range(N_CORES)))
    out = np.zeros((B, n_tok, D), np.float32)
    for core in range(N_CORES):
        b = core // N_GROUPS
        out[b] += res.results[core]["out_p"].astype(np.float32)
    out += np.asarray(inputs["b_o"], np.float32)
    return out
